# revision 7
# baseline (speedup 1.0000x reference)
"""DeepSpeedSelfAttention (LN + QKV + softmax-attention + out-proj) on 8 trn2 cores.

Sharding: core c -> (batch b = c//2, head-group g = c%2 of 8 heads).

v2: restructured for engine overlap. The attention phase is jointly
Scalar(EXP)- and Tensor-bound, so all projection work is pushed INTO that
window and the serial prefix/tail are minimized:
  - LayerNorm quarters pipelined with K/Q chunk-0 projections
  - pair-0 scores+exp emitted first, V tiles fill the PE while ACT
    starts the exp stream; per-kt Vg tiles give fine-grained deps
  - pairs hp=1..3 preceded by their K/Q chunk; out-projection of token
    chunk qq interleaved right after pair-3 finishes that qq
  - key/ctx leave the device feature-major bf16 (host transposes/casts)
  - softmax normalization reads PSUM directly (recip + broadcast + mul)
Outputs per core: key^T/ctx^T (bf16 chunk-major), value (f32 token-major),
out partial (f32); host sums the two per-batch out partials.
"""

import sys

for _p in ("/opt/trn_rl_repo", "/opt/trn_rl_repo/concourse"):
    if _p not in sys.path:
        sys.path.insert(0, _p)

import numpy as np
import ml_dtypes

import concourse.bass as bass
import concourse.tile as tile
from concourse import mybir, bacc
from concourse import bass_utils
from concourse.bass import ts

F32 = mybir.dt.float32
BF16 = mybir.dt.bfloat16
AF = mybir.ActivationFunctionType
ALU = mybir.AluOpType

B, S, H = 4, 2048, 1024
HEADS = 16
DH = H // HEADS          # 64
N_CORES = 8
HPC = HEADS // 2         # 8 heads per core
RW = HPC * DH            # 512 features per core
EPS = 1e-12
TT_N = S // 128          # 16 token tiles
DAUG = DH + 1            # 65: v features + denominator column


def build_program():
    nc = bacc.Bacc(trn_type="TRN2")

    x_d = nc.dram_tensor("x", [S, H], F32, kind="ExternalInput")
    wqT_d = nc.dram_tensor("wqT", [128, 8, RW], BF16, kind="ExternalInput")
    wkT_d = nc.dram_tensor("wkT", [128, 8, RW], BF16, kind="ExternalInput")
    wvT_d = nc.dram_tensor("wvT", [128, 8, RW], BF16, kind="ExternalInput")
    owT_d = nc.dram_tensor("owT", [128, 4, H], BF16, kind="ExternalInput")
    bq_d = nc.dram_tensor("bq", [128, 4], F32, kind="ExternalInput")
    bk_d = nc.dram_tensor("bk", [128, 4], F32, kind="ExternalInput")
    bvb_d = nc.dram_tensor("bvb", [128, RW], F32, kind="ExternalInput")
    expm_d = nc.dram_tensor("expm", [128, TT_N], F32, kind="ExternalInput")

    key_d = nc.dram_tensor("key_out", [128, 4, S], BF16, kind="ExternalOutput")
    val_d = nc.dram_tensor("value_out", [S, RW], F32, kind="ExternalOutput")
    ctx_d = nc.dram_tensor("ctx_out", [128, 4, S], BF16, kind="ExternalOutput")
    out_d = nc.dram_tensor("out_partial", [S, H], F32, kind="ExternalOutput")

    x_ap, key_ap, val_ap, ctx_ap, out_ap = (
        x_d.ap(), key_d.ap(), val_d.ap(), ctx_d.ap(), out_d.ap())

    with tile.TileContext(nc) as tc:
        with (
            tc.tile_pool(name="const", bufs=1) as cp,
            tc.tile_pool(name="persist", bufs=1) as pp,
            tc.tile_pool(name="psatt", bufs=2, space="PSUM") as ps_att,
            tc.tile_pool(name="psctx", bufs=1, space="PSUM") as ps_ctx,
            tc.tile_pool(name="psproj", bufs=2, space="PSUM") as ps_proj,
            tc.tile_pool(name="lnw", bufs=3) as lw,
            tc.tile_pool(name="aw", bufs=3) as aw,
            tc.tile_pool(name="ptp", bufs=4) as ptp,
        ):
            wq_s = cp.tile([128, 8, RW], BF16)
            nc.gpsimd.dma_start(out=wq_s, in_=wqT_d.ap())
            wk_s = cp.tile([128, 8, RW], BF16)
            nc.gpsimd.dma_start(out=wk_s, in_=wkT_d.ap())
            wv_s = cp.tile([128, 8, RW], BF16)
            nc.gpsimd.dma_start(out=wv_s, in_=wvT_d.ap())
            ow_s = cp.tile([128, 4, H], BF16)
            nc.gpsimd.dma_start(out=ow_s, in_=owT_d.ap())
            bq_s = cp.tile([128, 4], F32)
            nc.gpsimd.dma_start(out=bq_s, in_=bq_d.ap())
            bk_s = cp.tile([128, 4], F32)
            nc.gpsimd.dma_start(out=bk_s, in_=bk_d.ap())
            bvb_s = cp.tile([128, RW], F32)
            nc.gpsimd.dma_start(out=bvb_s, in_=bvb_d.ap())
            expm_s = cp.tile([128, TT_N], F32)
            nc.gpsimd.dma_start(out=expm_s, in_=expm_d.ap())
            eps_s = cp.tile([128, 1], F32)
            nc.vector.memset(eps_s, EPS)

            lnT4 = [pp.tile([128, 8, S // 4], BF16, tag=f"lnT{q}",
                            name=f"lnT{q}") for q in range(4)]
            qt = [pp.tile([128, S], BF16, tag=f"qt{rc}", name=f"qt{rc}")
                  for rc in range(4)]
            kt_ = [pp.tile([128, S], BF16, tag=f"kt{rc}", name=f"kt{rc}")
                   for rc in range(4)]
            Vg = [pp.tile([128, HPC, DAUG], BF16, tag=f"Vg{t}",
                          name=f"Vg{t}") for t in range(TT_N)]
            ctxT = [pp.tile([128, S], BF16, tag=f"ctxT{h}", name=f"ctxT{h}")
                    for h in range(4)]

            def qk_block(rc, tb, w_s, b_s, dst):
                ps = ps_proj.tile([128, 512], F32, tag="p",
                                  name=f"pqk{rc}{tb}{b_s.name}")
                for hc in range(8):
                    nc.tensor.matmul(
                        ps, w_s[:, hc, ts(rc, 128)], lnT4[tb][:, hc, :],
                        start=(hc == 0), stop=(hc == 7))
                nc.vector.tensor_scalar_add(
                    out=dst[:, ts(tb, 512)], in0=ps,
                    scalar1=b_s[:, rc:rc + 1])

            # ---------------- LayerNorm + lnT, K0/Q0 pipelined ----------
            for tt in range(TT_N):
                xt = lw.tile([128, H], F32, tag="xt")
                nc.scalar.dma_start(out=xt, in_=x_ap[ts(tt, 128), :])
                stats = lw.tile([128, 2, 6], F32, tag="st")
                nc.vector.bn_stats(out=stats[:, 0, :], in_=xt[:, 0:512])
                nc.vector.bn_stats(out=stats[:, 1, :], in_=xt[:, 512:1024])
                mv = lw.tile([128, 2], F32, tag="mv")
                nc.vector.bn_aggr(out=mv, in_=stats)
                sd = lw.tile([128, 1], F32, tag="sd")
                nc.scalar.activation(out=sd, in_=mv[:, 1:2], func=AF.Sqrt,
                                     bias=eps_s[:, 0:1], scale=1.0)
                rstd = lw.tile([128, 1], F32, tag="rstd")
                nc.vector.reciprocal(out=rstd, in_=sd)
                lnt = lw.tile([128, H], BF16, tag="lnt")
                nc.vector.tensor_scalar(out=lnt, in0=xt, scalar1=mv[:, 0:1],
                                        scalar2=rstd, op0=ALU.subtract,
                                        op1=ALU.mult)
                nc.sync.dma_start(out=lnT4[tt // 4][:, :, ts(tt % 4, 128)],
                                  in_=lnt, transpose=True)
                if tt % 4 == 3:
                    tb = tt // 4
                    qk_block(0, tb, wk_s, bk_s, kt_[0])
                    qk_block(0, tb, wq_s, bq_s, qt[0])
            nc.gpsimd.dma_start(out=key_ap[:, 0, :], in_=kt_[0])

            # ---------------- V tile (emitted later, under pair0 exps) --
            def v_tile(tt):
                ps = ps_proj.tile([128, 512], F32, tag="p", name=f"psv{tt}")
                for hc in range(8):
                    nc.tensor.matmul(
                        ps, lnT4[tt // 4][:, hc, ts(tt % 4, 128)],
                        wv_s[:, hc, :], start=(hc == 0), stop=(hc == 7))
                v1 = lw.tile([128, RW], F32, tag="v1")
                nc.vector.tensor_add(out=v1, in0=ps, in1=bvb_s)
                nc.gpsimd.dma_start(out=val_ap[ts(tt, 128), :], in_=v1)
                nc.vector.memset(Vg[tt], 0.0)
                v1h = v1[:].rearrange("p (h d) -> p h d", d=DH)
                nc.vector.tensor_scalar_mul(
                    out=Vg[tt][:, :, 0:DH], in0=v1h,
                    scalar1=expm_s[:, tt:tt + 1])
                nc.vector.tensor_scalar_add(
                    out=Vg[tt][:, :, DH:DAUG], in0=Vg[tt][:, :, DH:DAUG],
                    scalar1=expm_s[:, tt:tt + 1])

            # ---------------- attention: one (pair, q-chunk) ------------
            def emit_pv(hp, kt, pt, pc0, pc1):
                nc.tensor.matmul(pc0, Vg[kt][:, 2 * hp, :], pt[:, 0, :],
                                 start=(kt == 0), stop=(kt == TT_N - 1))
                nc.tensor.matmul(pc1, Vg[kt][:, 2 * hp + 1, :], pt[:, 1, :],
                                 start=(kt == 0), stop=(kt == TT_N - 1))

            def norm_store(hp, qq, pc0, pc1):
                for j, pc in ((0, pc0), (1, pc1)):
                    cu = aw.tile([DH, 512], F32, tag="cu")
                    nc.vector.tensor_copy(out=cu, in_=pc[0:DH, :])
                    den = aw.tile([1, 512], F32, tag="den")
                    nc.vector.tensor_copy(out=den, in_=pc[DH:DAUG, :])
                    rec = aw.tile([1, 512], F32, tag="rec")
                    nc.vector.reciprocal_approx_fast(out=rec, in_=den)
                    rbc = aw.tile([64, 512], F32, tag="rbc")
                    nc.gpsimd.partition_broadcast(rbc, rec)
                    nc.vector.tensor_mul(
                        out=ctxT[hp][64 * j:64 * (j + 1), ts(qq, 512)],
                        in0=cu, in1=rbc)

            def attn_qq(hp, qq, vfill=False):
                pc0 = ps_ctx.tile([DAUG, 512], F32, tag="c0",
                                  name=f"pc0_{hp}{qq}")
                pc1 = ps_ctx.tile([DAUG, 512], F32, tag="c1",
                                  name=f"pc1_{hp}{qq}")
                ps_tiles = {}

                def emit_sc(kt):
                    ps_s = ps_att.tile([128, 2, 512], F32, tag="s",
                                       name=f"pss{hp}{qq}{kt}")
                    ps_tiles[kt] = ps_s
                    nc.tensor.matmul(
                        ps_s[:, 0, :], kt_[hp][0:64, ts(kt, 128)],
                        qt[hp][0:64, ts(qq, 512)], start=True, stop=True)
                    nc.tensor.matmul(
                        ps_s[:, 1, :], kt_[hp][64:128, ts(kt, 128)],
                        qt[hp][64:128, ts(qq, 512)], start=True, stop=True,
                        tile_position=(64, 0))

                def emit_exp_pv(kt):
                    ps_s = ps_tiles.pop(kt)
                    pt = ptp.tile([128, 2, 512], BF16, tag="pt",
                                  name=f"pt{hp}{qq}{kt}")
                    nc.scalar.activation(out=pt, in_=ps_s, func=AF.Exp,
                                         scale=0.125)
                    if vfill:
                        v_tile(kt)
                    emit_pv(hp, kt, pt, pc0, pc1)

                for i in range(TT_N + 1):
                    if i < TT_N:
                        emit_sc(i)
                    if i >= 1:
                        emit_exp_pv(i - 1)
                norm_store(hp, qq, pc0, pc1)

            # ---------------- out projection for one q-chunk ------------
            def outproj_qq(qq):
                for tt4 in range(4):
                    tt = 4 * qq + tt4
                    pa = ps_proj.tile([128, 512], F32, tag="p",
                                      name=f"po0_{tt}")
                    pb = ps_proj.tile([128, 512], F32, tag="p",
                                      name=f"po1_{tt}")
                    for jc in range(4):
                        nc.tensor.matmul(pa, ctxT[jc][:, ts(tt, 128)],
                                         ow_s[:, jc, 0:512],
                                         start=(jc == 0), stop=(jc == 3))
                        nc.tensor.matmul(pb, ctxT[jc][:, ts(tt, 128)],
                                         ow_s[:, jc, 512:1024],
                                         start=(jc == 0), stop=(jc == 3))
                    ost = lw.tile([128, H], F32, tag="xt")
                    nc.vector.tensor_copy(out=ost[:, 0:512], in_=pa)
                    nc.vector.tensor_copy(out=ost[:, 512:1024], in_=pb)
                    nc.gpsimd.dma_start(out=out_ap[ts(tt, 128), :], in_=ost)

            # ---------------- main schedule -----------------------------
            # pair 0, qq 0: V tiles interleaved per-step into the pipeline
            attn_qq(0, 0, vfill=True)
            for qq in range(1, 4):
                attn_qq(0, qq)
            nc.gpsimd.dma_start(out=ctx_ap[:, 0, :], in_=ctxT[0])

            for rc in range(1, 4):
                for tb in range(4):
                    qk_block(rc, tb, wk_s, bk_s, kt_[rc])
                    qk_block(rc, tb, wq_s, bq_s, qt[rc])
                nc.gpsimd.dma_start(out=key_ap[:, rc, :], in_=kt_[rc])
                for qq in range(4):
                    attn_qq(rc, qq)
                    if rc == 3:
                        outproj_qq(qq)
                nc.gpsimd.dma_start(out=ctx_ap[:, rc, :], in_=ctxT[rc])

    nc.compile()
    return nc


def host_prep(inputs):
    """Build the 8 per-core input maps from full inputs."""
    x = np.asarray(inputs["input"], np.float32)
    mask = np.asarray(inputs["input_mask"], np.float32)
    norm_w = np.asarray(inputs["norm_w"], np.float32)
    norm_b = np.asarray(inputs["norm_b"], np.float32)
    qkvw = np.asarray(inputs["attn_qkvw"], np.float32)
    qkvb = np.asarray(inputs["attn_qkvb"], np.float32)
    ow = np.asarray(inputs["attn_ow"], np.float32)

    wfold = qkvw * norm_w[None, :]          # [3H, H]
    bfold = qkvb + qkvw @ norm_b            # [3H]

    in_maps = []
    for c in range(N_CORES):
        b, g = divmod(c, 2)
        rq = slice(g * RW, g * RW + RW)
        rk = slice(H + g * RW, H + g * RW + RW)
        rv = slice(2 * H + g * RW, 2 * H + g * RW + RW)

        def wT(rows):
            # [RW, H] -> W^T [H, RW] -> [128, 8, RW] with h = hc*128 + p
            w = wfold[rows].T.astype(ml_dtypes.bfloat16)
            return np.ascontiguousarray(
                w.reshape(8, 128, RW).transpose(1, 0, 2))

        def bcol(rows):
            # [RW] -> [128, 4] with r = rc*128 + p
            return np.ascontiguousarray(
                bfold[rows].reshape(4, 128).T.astype(np.float32))

        owT = ow[:, g * RW:g * RW + RW].T.astype(ml_dtypes.bfloat16)  # [RW, H]
        owT = np.ascontiguousarray(owT.reshape(4, 128, H).transpose(1, 0, 2))
        expm = np.exp(mask[b, 0, 0, :]).astype(np.float32)
        in_maps.append({
            "x": np.ascontiguousarray(x[b]),
            "wqT": wT(rq), "wkT": wT(rk), "wvT": wT(rv),
            "owT": owT,
            "bq": bcol(rq), "bk": bcol(rk),
            "bvb": np.ascontiguousarray(
                np.broadcast_to(bfold[rv][None, :], (128, RW)).astype(np.float32)),
            "expm": np.ascontiguousarray(expm.reshape(TT_N, 128).T),
        })
    return in_maps


_NC_CACHE = None


def run_cores(in_maps, **kwargs):
    global _NC_CACHE
    if _NC_CACHE is None:
        _NC_CACHE = build_program()
    return bass_utils.run_bass_kernel_spmd(
        _NC_CACHE, in_maps, core_ids=list(range(N_CORES)), **kwargs)


def assemble(results):
    out = np.zeros((B, S, H), np.float32)
    key = np.zeros((B, S, H), np.float32)
    val = np.zeros((B, S, H), np.float32)
    ctx = np.zeros((B, S, H), np.float32)
    for c in range(N_CORES):
        b, g = divmod(c, 2)
        cols = slice(g * RW, g * RW + RW)
        r = results[c]
        # key/ctx arrive feature-major [128, 4, S] bf16: f = rc*128 + p
        kT = np.asarray(r["key_out"], np.float32)      # [128, 4, S]
        key[b, :, cols] = kT.transpose(2, 1, 0).reshape(S, RW)
        cT = np.asarray(r["ctx_out"], np.float32)
        ctx[b, :, cols] = cT.transpose(2, 1, 0).reshape(S, RW)
        val[b, :, cols] = r["value_out"]
        out[b] += r["out_partial"]
    return out, key, val, ctx


def kernel(**inputs):
    in_maps = host_prep(inputs)
    res = run_cores(in_maps)
    return assemble(res.results)
